# revision 34
# baseline (speedup 1.0000x reference)
"""Trainium2 Bass kernel for nn_MultiHeadAttention_53463752900838.

Math (per batch element b, one NeuronCore each — pure data parallel over B=8):
  qkv = w_qkv @ x + b_qkv                     (3072, T)
  q,k,v per head h: (64, T);  q scaled by 1/8 (folded into weights on host)
  scores[t,h,g] = sum_d q[h,d,t] k[g,d,t]     per-timestep 16x16 Gram matrix
  attn = softmax over t  (per (h,g) pair)
  context[h,d,t] = sum_g attn[t,h,g] v[g,d,t]
  out = w_out @ context + b_out               (1024, T)

Layout strategy (all bf16 matmuls, fp32 PSUM):
  Phase 1 (per 256-t span): QKV projection in (o, t) orientation, bias folded
    into the PSUM evacuation (per-partition bias add on DVE/Act, no bias
    matmuls). Q/K marshaled by strided DMA into a [64, (kind, h, t)] tile
    (both at partition base 0). V evacuated into a full-T SBUF-resident
    tile (no DRAM spill). Per-t 16x16 scores matmuls write a 4-bank psum
    tile packed over partitions by 128-t blocks: partition (32*tc+g),
    free (slot*16+h) with t = grp*512 + tc*128 + slot. One fused-exp evac
    per 512 timesteps into the SBUF-resident E tile; running Z on DVE.
  Phase 2 (per 512-t chunk): attn = E * (1/Z) on DVE (2x mode, partition-
    aligned). Context computed TRANSPOSED per t with matching partition
    bases: lhsT = v_t[16 g, 64 d] and rhs = attn_t[16 g, 8 h] both at base
    32*tc, out at psum partitions (64*hp + d) via tile_position; cheap evac
    to channel-major CTC; output projection accumulates over 8 head-pairs,
    bias via activation evac, written straight to (C, T) f32 output.
"""

import os
import sys
import contextlib

import numpy as np
import ml_dtypes

for p in ("/opt/trn_rl_repo",):
    if p not in sys.path and os.path.isdir(p):
        sys.path.insert(0, p)

import concourse.bass as bass
import concourse.tile as tile
from concourse import mybir
from concourse.bass_utils import run_bass_kernel_spmd

F32 = mybir.dt.float32
BF16 = mybir.dt.bfloat16

N_CORES = 8
C = 1024
H = 16
DK = 64
OC3 = 3072


def _split_sync_waits(nc, limit=1):
    """walrus codegen rejects too many semaphore waits per instruction (CTRL
    class takes 1); hoist overflow waits onto NoOps inserted before the
    offending instruction."""
    counter = [0]
    n_split = 0
    for fn in nc.m.functions:
        for bb in fn.blocks:
            out = []
            for ins in bb.instructions:
                si = getattr(ins, "sync_info", None)
                waits = list(si.on_wait) if (si is not None and si.on_wait) else []
                if len(waits) > limit:
                    n_split += 1
                    extra, keep = waits[:-limit], waits[-limit:]
                    for i in range(0, len(extra), limit):
                        counter[0] += 1
                        out.append(
                            mybir.InstNoOp(
                                name=f"I-wsplit-{counter[0]}",
                                opcode="NoOp",
                                engine=ins.engine,
                                ins=[],
                                outs=[],
                                sync_info=mybir.SyncInfo(
                                    on_wait=list(extra[i : i + limit]), on_update=[]
                                ),
                            )
                        )
                    si.on_wait = keep
                out.append(ins)
            bb.instructions[:] = out
    return n_split


def build_kernel(T=4096):
    S1 = 256              # phase-1 span (t per QKV round)
    NS1 = T // S1         # 16
    G = 512               # scores/phase-2 group size
    NG = T // G           # 8
    nc = bass.Bass("TRN2", target_bir_lowering=False, debug=False)

    x_in = nc.dram_tensor("x", [C, T], BF16, kind="ExternalInput").ap()
    wq_in = nc.dram_tensor("wqT", [C, OC3], BF16, kind="ExternalInput").ap()
    bqc_in = nc.dram_tensor("bqc", [128, 24], F32, kind="ExternalInput").ap()
    wo_in = nc.dram_tensor("wo2", [128, 8 * C], BF16, kind="ExternalInput").ap()
    boc_in = nc.dram_tensor("boc", [128, 8], F32, kind="ExternalInput").ap()
    out_d = nc.dram_tensor("out", [C, T], F32, kind="ExternalOutput").ap()

    Exp = mybir.ActivationFunctionType.Exp
    Ident = mybir.ActivationFunctionType.Identity
    Copy = mybir.ActivationFunctionType.Copy
    ADD = mybir.AluOpType.add
    MUL = mybir.AluOpType.mult
    AX = mybir.AxisListType.X

    with tile.TileContext(nc) as tc, contextlib.ExitStack() as octx:
        const = octx.enter_context(tc.tile_pool(name="const", bufs=1))
        bqc_sb = const.tile([128, 24], F32, tag="bqc")
        nc.sync.dma_start(bqc_sb[:], bqc_in)
        boc_sb = const.tile([128, 8], F32, tag="boc")
        nc.sync.dma_start(boc_sb[:], boc_in)
        # SBUF-resident V: [128=(gp*64+d), (j=8, T)] for g = 2j+gp
        v_res = const.tile([128, 8 * T], BF16, tag="vres")
        # SBUF-resident exp(scores): [128=(32*tc+g), (grp, slot, h)]
        # with t = grp*512 + tc*128 + slot
        se_res = const.tile([128, 4 * T], BF16, tag="seres")
        zfin = const.tile([16, 8 * 64], F32, tag="zfin")  # (grp, tc, h)
        zred = const.tile([16, 16], F32, tag="zred")
        rrecf = const.tile([16, 16], F32, tag="rrecf")
        rrec4 = const.tile([128, 16], BF16, tag="rrec4")
        nc.vector.memset(rrec4[:], 0.0)

        # ---------------- PHASE 1: QKV + scores + exp + Z ----------------
        with contextlib.ExitStack() as ctx:
            wpool = ctx.enter_context(tc.tile_pool(name="wq", bufs=1))
            xpool = ctx.enter_context(tc.tile_pool(name="xp", bufs=2))

            def load_x(s):
                xa = xpool.tile([128, 8 * S1], BF16, tag="xa")
                nc.sync.dma_start(
                    xa[:].rearrange("p (k t) -> p k t", k=8),
                    x_in[:, s * S1 : (s + 1) * S1].rearrange(
                        "(k p) t -> p k t", k=8
                    ),
                )
                return xa

            xa_next = load_x(0)
            # wq loaded in 8 column chunks of 3 mtiles each so the first
            # mtiles can start a few us in instead of waiting for all 6 MB;
            # the first chunk is split across SWDGE and HWDGE paths.
            CW = OC3 // 8  # 384
            wq_sb = []  # [chunk][k] -> [128, 384]
            for cch in range(8):
                row = []
                for k in range(8):
                    w = wpool.tile([128, CW], BF16, tag=f"wq{cch}_{k}")
                    eng = nc.gpsimd if (cch == 0 and k % 2 == 0) else nc.sync
                    eng.dma_start(
                        w[:], wq_in[k * 128 : (k + 1) * 128, cch * CW : (cch + 1) * CW]
                    )
                    row.append(w)
                wq_sb.append(row)

            stpool = ctx.enter_context(tc.tile_pool(name="st", bufs=2))
            qkpool = ctx.enter_context(tc.tile_pool(name="qkt", bufs=2))
            zpool = ctx.enter_context(tc.tile_pool(name="zp", bufs=2))
            ps_a = ctx.enter_context(tc.tile_pool(name="psA", bufs=4, space="PSUM"))
            ps_s = ctx.enter_context(tc.tile_pool(name="psS", bufs=1, space="PSUM"))

            # zero-fill the scores psum buffer once: partitions 32*tc+16..31
            # are never written by the 16-col matmuls but are read by the
            # wide evac.
            pstmp = ps_s.tile([128, 2048], F32, tag="psS")
            nc.vector.memset(pstmp[:], 0.0)

            def emit_scores(grp, qkts):
                # qkts: two qkt tiles covering spans (2*grp, 2*grp+1); each is
                # [64, (kind=2, h=16, t=S1)], kind 0 = Q, kind 1 = K.
                pss = ps_s.tile([128, 2048], F32, tag="psS")
                for tc4 in range(4):
                    qkt = qkts[tc4 >> 1]
                    qv = qkt[:].rearrange("d (kd h t) -> d kd h t", kd=2, h=H)
                    for sl in range(128):
                        t = (tc4 & 1) * 128 + sl
                        nc.tensor.matmul(
                            pss[32 * tc4 : 32 * tc4 + 16, sl * 16 : sl * 16 + 16],
                            lhsT=qv[:, 1, :, t],
                            rhs=qv[:, 0, :, t],
                            start=True,
                            stop=True,
                            tile_position=(0, 32 * tc4),
                        )
                seg = se_res[:, grp * 2048 : (grp + 1) * 2048]
                nc.scalar.activation(seg, pss[:], Exp)
                zt = zpool.tile([128, 16], F32, tag="zt")
                nc.vector.tensor_reduce(
                    zt[:],
                    seg.rearrange("p (sl h) -> p h sl", h=H),
                    axis=AX,
                    op=ADD,
                )
                for tc4 in range(4):
                    # Act queue: never head-of-line blocks the SP/Pool marshal
                    # DMAs at the phase boundary
                    nc.scalar.dma_start(
                        zfin[
                            0:16, grp * 64 + tc4 * 16 : grp * 64 + (tc4 + 1) * 16
                        ],
                        zt[32 * tc4 : 32 * tc4 + 16, :],
                    )

            pending = []          # qkt tiles not yet consumed by emit_scores
            ngrp_done = 0
            for s in range(NS1):
                xa = xa_next
                st = stpool.tile([128, 16 * S1], BF16, tag="st")
                for m in range(24):
                    ps = ps_a.tile([128, S1], F32, tag="psA")
                    for k in range(8):
                        nc.tensor.matmul(
                            ps[:],
                            lhsT=wq_sb[m // 3][k][
                                :, (m % 3) * 128 : (m % 3 + 1) * 128
                            ],
                            rhs=xa[:, k * S1 : (k + 1) * S1],
                            start=(k == 0),
                            stop=(k == 7),
                        )
                    if m < 16:
                        dst = st[:, m * S1 : (m + 1) * S1]
                    else:
                        j = m - 16
                        dst = v_res[:, j * T + s * S1 : j * T + (s + 1) * S1]
                    if m % 2 == 0:
                        nc.vector.tensor_scalar(
                            dst, ps[:], bqc_sb[:, m : m + 1], None, ADD
                        )
                    else:
                        nc.scalar.activation(
                            dst, ps[:], Ident, bias=bqc_sb[:, m : m + 1]
                        )
                    # interleave scores of the previous group so the PE never
                    # waits on marshal DMAs
                    if m == 15 and len(pending) == 2 and s % 2 == 0:
                        emit_scores(ngrp_done, pending)
                        pending = []
                        ngrp_done += 1
                    if m == 11 and s + 1 < NS1:
                        xa_next = load_x(s + 1)
                    # marshal Q (m 0-7) as soon as its evacs are emitted, K
                    # (m 8-15) right after; qkt free = (kd*16+2*mm+hp)*S1+t =
                    # m*(2*S1) + hp*S1 + t with stage free (m, t).
                    if m == 7:
                        qkt = qkpool.tile([64, 2 * H * S1], BF16, tag="qkt")
                    if m in (7, 15):
                        mlo = 0 if m == 7 else 8
                        for hp in range(2):
                            nc.sync.dma_start(
                                qkt[:].rearrange(
                                    "d (m hp t) -> hp d m t", m=16, hp=2
                                )[hp, :, mlo : mlo + 8, :],
                                st[hp * 64 : (hp + 1) * 64, :].rearrange(
                                    "d (m t) -> d m t", m=16
                                )[:, mlo : mlo + 8, :],
                            )
                pending.append(qkt)
            while pending:
                emit_scores(ngrp_done, pending[:2])
                pending = pending[2:]
                ngrp_done += 1

        # ---------------- PHASE 2: attn * V + output projection ----------------
        with contextlib.ExitStack() as ctx:
            wopool = ctx.enter_context(tc.tile_pool(name="wo", bufs=1))
            wo_sb = wopool.tile([128, 8 * C], BF16, tag="wo")

            vtpool = ctx.enter_context(tc.tile_pool(name="vt", bufs=2))
            atpool = ctx.enter_context(tc.tile_pool(name="at", bufs=2))
            ctcpool = ctx.enter_context(tc.tile_pool(name="ctc", bufs=2))
            opool = ctx.enter_context(tc.tile_pool(name="osb", bufs=3))
            ps_c = ctx.enter_context(tc.tile_pool(name="psC", bufs=3, space="PSUM"))
            ps_o = ctx.enter_context(tc.tile_pool(name="psO", bufs=2, space="PSUM"))

            def marshal_vt(grp, load_wo=False):
                # vt4[32*tc + 2j+gp, (d, slot)] = v[g=2j+gp, d,
                # t = grp*512 + tc*128 + slot]
                vt4 = vtpool.tile([128, DK * 128], BF16, tag="vt4")
                for tc4 in range(4):
                    for j in range(8):
                        dst = vt4[
                            32 * tc4 + 2 * j : 32 * tc4 + 2 * j + 2, :
                        ].rearrange("p (d sl) -> p d sl", d=DK)
                        src = v_res[
                            :,
                            j * T
                            + grp * G
                            + tc4 * 128 : j * T
                            + grp * G
                            + (tc4 + 1) * 128,
                        ]
                        eng = (nc.gpsimd, nc.sync, nc.scalar, nc.gpsimd,
                               nc.sync, nc.scalar, nc.gpsimd, nc.sync)[j]
                        eng.dma_start(dst, src)
                    if load_wo and tc4 > 0:
                        # split the 2 MB wo load so it never monopolizes the
                        # DMA engines during the phase-boundary critical chain
                        nc.sync.dma_start(
                            wo_sb[:, (tc4 - 1) * 2048 : tc4 * 2048],
                            wo_in[:, (tc4 - 1) * 2048 : tc4 * 2048],
                        )
                if load_wo:
                    nc.sync.dma_start(wo_sb[:, 3 * 2048 :], wo_in[:, 3 * 2048 :])
                return vt4

            # start grp-0 V marshal DMAs immediately at the phase boundary;
            # the Z-finalize chain below runs concurrently on other engines.
            vt_cur = marshal_vt(0, load_wo=True)

            # ---- finalize Z -> rrec4 ----
            nc.vector.tensor_reduce(
                zred[:],
                zfin[0:16, :].rearrange("g (gt h) -> g h gt", h=H),
                axis=AX,
                op=ADD,
            )
            nc.vector.reciprocal(rrecf[:], zred[:])
            nc.vector.tensor_copy(rrec4[0:16, :], rrecf[:])
            for tc4 in range(1, 4):
                nc.scalar.dma_start(
                    rrec4[32 * tc4 : 32 * tc4 + 16, :], rrec4[0:16, :]
                )

            def emit_outproj(ctc, grp):
                for ob2 in range(4):
                    osb = opool.tile([128, 2 * G], F32, tag="osb")
                    for obh in range(2):
                        ob = 2 * ob2 + obh
                        pso = ps_o.tile([128, G], F32, tag="psO")
                        for m in range(8):
                            nc.tensor.matmul(
                                pso[:],
                                lhsT=wo_sb[
                                    :, m * C + ob * 128 : m * C + (ob + 1) * 128
                                ],
                                rhs=ctc[:, m * G : (m + 1) * G],
                                start=(m == 0),
                                stop=(m == 7),
                            )
                        nc.scalar.activation(
                            osb[:, obh * G : (obh + 1) * G],
                            pso[:],
                            Ident,
                            bias=boc_sb[:, ob : ob + 1],
                        )
                    nc.sync.dma_start(
                        out_d[
                            ob2 * 256 : (ob2 + 1) * 256, grp * G : (grp + 1) * G
                        ].rearrange("(obh p) t -> p obh t", obh=2),
                        osb[:].rearrange("p (obh t) -> p obh t", obh=2),
                    )

            def emit_at(grp):
                # attn = E * (1/Z)
                at4 = atpool.tile([128, 2048], BF16, tag="at4")
                nc.vector.tensor_tensor(
                    out=at4[:].rearrange("p (sl h) -> p sl h", h=H),
                    in0=se_res[:, grp * 2048 : (grp + 1) * 2048].rearrange(
                        "p (sl h) -> p sl h", h=H
                    ),
                    in1=rrec4[:].unsqueeze(1).broadcast_to([128, 128, H]),
                    op=MUL,
                )
                return at4

            at_cur = emit_at(0)
            pend = None
            for grp in range(NG):
                vt4 = vt_cur
                at4 = at_cur
                if grp + 1 < NG:
                    vt_cur = marshal_vt(grp + 1)
                ctc = ctcpool.tile([128, 8 * G], BF16, tag="ctc")
                vtv = vt4[:].rearrange("p (d sl) -> p sl d", d=DK)
                a4v = at4[:].rearrange("p (sl h) -> p sl h", h=H)
                for q in range(8):
                    pcs = ps_c.tile([128, 512], F32, tag="psC")
                    for i64 in range(64):
                        i = q * 64 + i64
                        tc4, sl = i >> 7, i & 127
                        lw = vtv[32 * tc4 : 32 * tc4 + 16, sl, :]
                        for hp in range(2):
                            nc.tensor.matmul(
                                pcs[64 * hp : 64 * hp + 64, i64 * 8 : i64 * 8 + 8],
                                lhsT=lw,
                                rhs=a4v[
                                    32 * tc4 : 32 * tc4 + 16, sl, 8 * hp : 8 * hp + 8
                                ],
                                start=True,
                                stop=True,
                                tile_position=(32 * tc4, 64 * hp),
                            )
                    dst = ctc[:].rearrange("p (m t) -> p m t", m=8)[
                        :, :, q * 64 : (q + 1) * 64
                    ]
                    src = pcs[:].rearrange("p (sc m) -> p m sc", m=8)
                    if q % 2 == 0:
                        nc.vector.tensor_copy(dst, src)
                    else:
                        nc.scalar.activation(dst, src, Copy)
                if grp + 1 < NG:
                    at_cur = emit_at(grp + 1)
                if pend is not None:
                    emit_outproj(*pend)
                pend = (ctc, grp)
            emit_outproj(*pend)

    _split_sync_waits(nc, limit=1)
    return nc


_NC_CACHE = {}


def _get_nc(T, _span=None):
    key = T
    if key not in _NC_CACHE:
        _NC_CACHE[key] = build_kernel(T)
    return _NC_CACHE[key]


def _prep_weights(w_qkv, b_qkv, w_out, b_out):
    bf = ml_dtypes.bfloat16
    w3 = w_qkv.reshape(H, 192, C).astype(np.float32)
    qw = (w3[:, :DK, :] / 8.0).reshape(H * DK, C)
    kw = w3[:, DK : 2 * DK, :].reshape(H * DK, C)
    vw = w3[:, 2 * DK :, :].reshape(H * DK, C)
    # mtile order: m 0..7 = Q head pairs, 8..15 = K head pairs, 16..23 = V.
    wqT = np.concatenate([qw, kw, vw], axis=0).T.copy().astype(bf)  # (C, 3072)
    b3 = b_qkv.reshape(H, 192).astype(np.float32)
    bq_all = np.concatenate(
        [
            (b3[:, :DK] / 8.0).reshape(-1),
            b3[:, DK : 2 * DK].reshape(-1),
            b3[:, 2 * DK :].reshape(-1),
        ]
    )
    bqc = bq_all.reshape(24, 128).T.copy().astype(np.float32)  # [128, 24]
    # wo2[hp*64+d, m*C+o] = w_out[o, (hp*8+m)*64+d]
    wom = w_out.astype(np.float32).reshape(C, H, DK)  # [o, h, d]
    w4 = wom.transpose(1, 2, 0).reshape(2, 8, DK, C)  # [hp, m, d, o]
    wo2 = np.ascontiguousarray(w4.transpose(0, 2, 1, 3)).reshape(128, 8 * C).astype(bf)
    boc = np.ascontiguousarray(b_out.astype(np.float32).reshape(8, 128).T)  # [128, 8]
    return wqT, bqc, wo2, boc


def kernel(x, w_qkv, b_qkv, w_out, b_out, _trace=False, _span=None):
    B, _, T = x.shape
    assert B == N_CORES
    nc = _get_nc(T)
    wqT, bqc, wo2, boc = _prep_weights(w_qkv, b_qkv, w_out, b_out)
    bf = ml_dtypes.bfloat16
    in_maps = []
    for b in range(B):
        in_maps.append(
            {
                "x": x[b].astype(bf),
                "wqT": wqT,
                "bqc": bqc,
                "wo2": wo2,
                "boc": boc,
            }
        )
    res = run_bass_kernel_spmd(nc, in_maps, list(range(N_CORES)), trace=_trace)
    out = np.stack([res.results[b]["out"] for b in range(B)], axis=0)
    if _trace:
        kernel.last_exec_time_ns = res.exec_time_ns
        kernel.last_results = res
    return out.astype(np.float32)


# revision 40
# speedup vs baseline: 1.0014x; 1.0014x over previous
"""Trainium2 Bass kernel for nn_MultiHeadAttention_53463752900838.

Math (per batch element b, one NeuronCore each — pure data parallel over B=8):
  qkv = w_qkv @ x + b_qkv                     (3072, T)
  q,k,v per head h: (64, T);  q scaled by 1/8 (folded into weights on host)
  scores[t,h,g] = sum_d q[h,d,t] k[g,d,t]     per-timestep 16x16 Gram matrix
  attn = softmax over t  (per (h,g) pair)
  context[h,d,t] = sum_g attn[t,h,g] v[g,d,t]
  out = w_out @ context + b_out               (1024, T)

Layout strategy (all bf16 matmuls, fp32 PSUM):
  Phase 1 (per 256-t span): QKV projection in (o, t) orientation, bias folded
    into the PSUM evacuation (per-partition bias add on DVE/Act, no bias
    matmuls). Q/K marshaled by strided DMA into a [64, (kind, h, t)] tile
    (both at partition base 0). V evacuated into a full-T SBUF-resident
    tile (no DRAM spill). Per-t 16x16 scores matmuls write a 4-bank psum
    tile packed over partitions by 128-t blocks: partition (32*tc+g),
    free (slot*16+h) with t = grp*512 + tc*128 + slot. One fused-exp evac
    per 512 timesteps into the SBUF-resident E tile; running Z on DVE.
  Phase 2 (per 512-t chunk): attn = E * (1/Z) on DVE (2x mode, partition-
    aligned). Context computed TRANSPOSED per t with matching partition
    bases: lhsT = v_t[16 g, 64 d] and rhs = attn_t[16 g, 8 h] both at base
    32*tc, out at psum partitions (64*hp + d) via tile_position; cheap evac
    to channel-major CTC; output projection accumulates over 8 head-pairs,
    bias via activation evac, written straight to (C, T) f32 output.
"""

import os
import sys
import contextlib

import numpy as np
import ml_dtypes

for p in ("/opt/trn_rl_repo",):
    if p not in sys.path and os.path.isdir(p):
        sys.path.insert(0, p)

import concourse.bass as bass
import concourse.tile as tile
from concourse import mybir
from concourse.bass_utils import run_bass_kernel_spmd

F32 = mybir.dt.float32
BF16 = mybir.dt.bfloat16

N_CORES = 8
C = 1024
H = 16
DK = 64
OC3 = 3072


def _split_sync_waits(nc, limit=1):
    """walrus codegen rejects too many semaphore waits per instruction (CTRL
    class takes 1); hoist overflow waits onto NoOps inserted before the
    offending instruction."""
    counter = [0]
    n_split = 0
    for fn in nc.m.functions:
        for bb in fn.blocks:
            out = []
            for ins in bb.instructions:
                si = getattr(ins, "sync_info", None)
                waits = list(si.on_wait) if (si is not None and si.on_wait) else []
                if len(waits) > limit:
                    n_split += 1
                    extra, keep = waits[:-limit], waits[-limit:]
                    for i in range(0, len(extra), limit):
                        counter[0] += 1
                        out.append(
                            mybir.InstNoOp(
                                name=f"I-wsplit-{counter[0]}",
                                opcode="NoOp",
                                engine=ins.engine,
                                ins=[],
                                outs=[],
                                sync_info=mybir.SyncInfo(
                                    on_wait=list(extra[i : i + limit]), on_update=[]
                                ),
                            )
                        )
                    si.on_wait = keep
                out.append(ins)
            bb.instructions[:] = out
    return n_split


def build_kernel(T=4096):
    S1 = 256              # phase-1 span (t per QKV round)
    NS1 = T // S1         # 16
    G = 512               # scores/phase-2 group size
    NG = T // G           # 8
    nc = bass.Bass("TRN2", target_bir_lowering=False, debug=False)

    x_in = nc.dram_tensor("x", [C, T], BF16, kind="ExternalInput").ap()
    wq_in = nc.dram_tensor("wqT", [C, OC3], BF16, kind="ExternalInput").ap()
    bqc_in = nc.dram_tensor("bqc", [128, 24], F32, kind="ExternalInput").ap()
    wo_in = nc.dram_tensor("wo2", [128, 8 * C], BF16, kind="ExternalInput").ap()
    boc_in = nc.dram_tensor("boc", [128, 8], F32, kind="ExternalInput").ap()
    out_d = nc.dram_tensor("out", [C, T], F32, kind="ExternalOutput").ap()

    Exp = mybir.ActivationFunctionType.Exp
    Ident = mybir.ActivationFunctionType.Identity
    Copy = mybir.ActivationFunctionType.Copy
    ADD = mybir.AluOpType.add
    MUL = mybir.AluOpType.mult
    AX = mybir.AxisListType.X

    with tile.TileContext(nc) as tc, contextlib.ExitStack() as octx:
        const = octx.enter_context(tc.tile_pool(name="const", bufs=1))
        bqc_sb = const.tile([128, 24], F32, tag="bqc")
        nc.sync.dma_start(bqc_sb[:], bqc_in)
        boc_sb = const.tile([128, 8], F32, tag="boc")
        nc.sync.dma_start(boc_sb[:], boc_in)
        # SBUF-resident V: [128=(gp*64+d), (j=8, T)] for g = 2j+gp
        v_res = const.tile([128, 8 * T], BF16, tag="vres")
        # SBUF-resident exp(scores): [128=(32*tc+g), (grp, slot, h)]
        # with t = grp*512 + tc*128 + slot
        se_res = const.tile([128, 4 * T], BF16, tag="seres")
        zfin = const.tile([16, 8 * 64], F32, tag="zfin")  # (grp, tc, h)
        zred = const.tile([16, 16], F32, tag="zred")
        rrecf = const.tile([16, 16], F32, tag="rrecf")
        rrec4 = const.tile([128, 16], BF16, tag="rrec4")
        nc.vector.memset(rrec4[:], 0.0)

        # ---------------- PHASE 1: QKV + scores + exp + Z ----------------
        with contextlib.ExitStack() as ctx:
            wpool = ctx.enter_context(tc.tile_pool(name="wq", bufs=1))
            xpool = ctx.enter_context(tc.tile_pool(name="xp", bufs=2))

            def load_x(s):
                xa = xpool.tile([128, 8 * S1], BF16, tag="xa")
                nc.sync.dma_start(
                    xa[:].rearrange("p (k t) -> p k t", k=8),
                    x_in[:, s * S1 : (s + 1) * S1].rearrange(
                        "(k p) t -> p k t", k=8
                    ),
                )
                return xa

            xa_next = load_x(0)
            # wq loaded in 8 column chunks of 3 mtiles each so the first
            # mtiles can start a few us in instead of waiting for all 6 MB;
            # the first chunk is split across SWDGE and HWDGE paths.
            CW = OC3 // 8  # 384
            wq_sb = []  # [chunk][k] -> [128, 384]
            for cch in range(8):
                row = []
                for k in range(8):
                    w = wpool.tile([128, CW], BF16, tag=f"wq{cch}_{k}")
                    eng = nc.gpsimd if (cch == 0 and k % 2 == 0) else nc.sync
                    eng.dma_start(
                        w[:], wq_in[k * 128 : (k + 1) * 128, cch * CW : (cch + 1) * CW]
                    )
                    row.append(w)
                wq_sb.append(row)

            stpool = ctx.enter_context(tc.tile_pool(name="st", bufs=2))
            qkpool = ctx.enter_context(tc.tile_pool(name="qkt", bufs=2))
            zpool = ctx.enter_context(tc.tile_pool(name="zp", bufs=3))
            ps_a = ctx.enter_context(tc.tile_pool(name="psA", bufs=4, space="PSUM"))
            ps_s = ctx.enter_context(tc.tile_pool(name="psS", bufs=1, space="PSUM"))

            # zero-fill the scores psum buffer once: partitions 32*tc+16..31
            # are never written by the 16-col matmuls but are read by the
            # wide evac.
            pstmp = ps_s.tile([128, 2048], F32, tag="psS")
            nc.vector.memset(pstmp[:], 0.0)

            def emit_scores(grp, qkts):
                # qkts: two qkt tiles covering spans (2*grp, 2*grp+1); each is
                # [64, (kind=2, h=16, t=S1)], kind 0 = Q, kind 1 = K.
                pss = ps_s.tile([128, 2048], F32, tag="psS")
                for tc4 in range(4):
                    qkt = qkts[tc4 >> 1]
                    qv = qkt[:].rearrange("d (kd h t) -> d kd h t", kd=2, h=H)
                    for sl in range(128):
                        t = (tc4 & 1) * 128 + sl
                        nc.tensor.matmul(
                            pss[32 * tc4 : 32 * tc4 + 16, sl * 16 : sl * 16 + 16],
                            lhsT=qv[:, 1, :, t],
                            rhs=qv[:, 0, :, t],
                            start=True,
                            stop=True,
                            tile_position=(0, 32 * tc4),
                        )
                # exp + Z-reduce in two halves so the tail-group critical
                # chain (exp -> reduce -> zfin -> ... -> first context matmul)
                # pipelines instead of serializing
                zth = [
                    zpool.tile([128, 16], F32, tag=f"zt{i}", name=f"zt{i}")
                    for i in range(2)
                ]
                for i in range(2):
                    seg = se_res[
                        :, grp * 2048 + i * 1024 : grp * 2048 + (i + 1) * 1024
                    ]
                    nc.scalar.activation(seg, pss[:, i * 1024 : (i + 1) * 1024], Exp)
                    nc.vector.tensor_reduce(
                        zth[i][:],
                        seg.rearrange("p (sl h) -> p h sl", h=H),
                        axis=AX,
                        op=ADD,
                    )
                zt = zpool.tile([128, 16], F32, tag="zts")
                nc.vector.tensor_tensor(
                    out=zt[:], in0=zth[0][:], in1=zth[1][:], op=ADD
                )
                last = grp == NG - 1
                for tc4 in range(4):
                    # Act (+SP for the last group): never head-of-line block
                    # the Pool marshal DMAs at the phase boundary
                    eng = (nc.scalar, nc.sync)[tc4 % 2] if last else nc.scalar
                    eng.dma_start(
                        zfin[
                            0:16, grp * 64 + tc4 * 16 : grp * 64 + (tc4 + 1) * 16
                        ],
                        zt[32 * tc4 : 32 * tc4 + 16, :],
                    )

            pending = []          # qkt tiles not yet consumed by emit_scores
            ngrp_done = 0
            for s in range(NS1):
                xa = xa_next
                st = stpool.tile([128, 16 * S1], BF16, tag="st")
                for m in range(24):
                    ps = ps_a.tile([128, S1], F32, tag="psA")
                    for k in range(8):
                        nc.tensor.matmul(
                            ps[:],
                            lhsT=wq_sb[m // 3][k][
                                :, (m % 3) * 128 : (m % 3 + 1) * 128
                            ],
                            rhs=xa[:, k * S1 : (k + 1) * S1],
                            start=(k == 0),
                            stop=(k == 7),
                        )
                    if m < 16:
                        dst = st[:, m * S1 : (m + 1) * S1]
                    else:
                        j = m - 16
                        dst = v_res[:, j * T + s * S1 : j * T + (s + 1) * S1]
                    if m % 2 == 0:
                        nc.vector.tensor_scalar(
                            dst, ps[:], bqc_sb[:, m : m + 1], None, ADD
                        )
                    else:
                        nc.scalar.activation(
                            dst, ps[:], Ident, bias=bqc_sb[:, m : m + 1]
                        )
                    # interleave scores of the previous group so the PE never
                    # waits on marshal DMAs
                    if m == 17 and len(pending) == 2 and s % 2 == 0:
                        emit_scores(ngrp_done, pending)
                        pending = []
                        ngrp_done += 1
                    if m == 11 and s + 1 < NS1:
                        xa_next = load_x(s + 1)
                    # marshal Q (m 0-7) as soon as its evacs are emitted, K
                    # (m 8-15) right after; qkt free = (kd*16+2*mm+hp)*S1+t =
                    # m*(2*S1) + hp*S1 + t with stage free (m, t).
                    if m == 7:
                        qkt = qkpool.tile([64, 2 * H * S1], BF16, tag="qkt")
                    if m in (7, 15):
                        mlo = 0 if m == 7 else 8
                        for hp in range(2):
                            nc.sync.dma_start(
                                qkt[:].rearrange(
                                    "d (m hp t) -> hp d m t", m=16, hp=2
                                )[hp, :, mlo : mlo + 8, :],
                                st[hp * 64 : (hp + 1) * 64, :].rearrange(
                                    "d (m t) -> d m t", m=16
                                )[:, mlo : mlo + 8, :],
                            )
                pending.append(qkt)
            while pending:
                emit_scores(ngrp_done, pending[:2])
                pending = pending[2:]
                ngrp_done += 1

        # ---------------- PHASE 2: attn * V + output projection ----------------
        with contextlib.ExitStack() as ctx:
            wopool = ctx.enter_context(tc.tile_pool(name="wo", bufs=1))
            wo_sb = wopool.tile([128, 8 * C], BF16, tag="wo")

            vtpool = ctx.enter_context(tc.tile_pool(name="vt", bufs=3))
            atpool = ctx.enter_context(tc.tile_pool(name="at", bufs=2))
            ctcpool = ctx.enter_context(tc.tile_pool(name="ctc", bufs=2))
            opool = ctx.enter_context(tc.tile_pool(name="osb", bufs=3))
            ps_c = ctx.enter_context(tc.tile_pool(name="psC", bufs=3, space="PSUM"))
            ps_o = ctx.enter_context(tc.tile_pool(name="psO", bufs=2, space="PSUM"))

            def marshal_vt(grp, load_wo=False):
                # vt4[32*tc + 2j+gp, (d, slot)] = v[g=2j+gp, d,
                # t = grp*512 + tc*128 + slot]
                vt4 = vtpool.tile([128, DK * 128], BF16, tag="vt4")
                for tc4 in range(4):
                    for j in range(8):
                        dst = vt4[
                            32 * tc4 + 2 * j : 32 * tc4 + 2 * j + 2, :
                        ].rearrange("p (d sl) -> p d sl", d=DK)
                        src = v_res[
                            :,
                            j * T
                            + grp * G
                            + tc4 * 128 : j * T
                            + grp * G
                            + (tc4 + 1) * 128,
                        ]
                        eng = (nc.gpsimd, nc.sync, nc.scalar, nc.gpsimd,
                               nc.sync, nc.scalar, nc.gpsimd, nc.sync)[j]
                        eng.dma_start(dst, src)
                    if load_wo and tc4 > 0:
                        # split the 2 MB wo load so it never monopolizes the
                        # DMA engines during the phase-boundary critical chain
                        nc.sync.dma_start(
                            wo_sb[:, (tc4 - 1) * 2048 : tc4 * 2048],
                            wo_in[:, (tc4 - 1) * 2048 : tc4 * 2048],
                        )
                if load_wo:
                    nc.sync.dma_start(wo_sb[:, 3 * 2048 :], wo_in[:, 3 * 2048 :])
                return vt4

            # ---- finalize Z -> rrec4 (emitted FIRST so its instructions sit
            # at the head of every queue at the phase boundary) ----
            nc.vector.tensor_reduce(
                zred[:],
                zfin[0:16, :].rearrange("g (gt h) -> g h gt", h=H),
                axis=AX,
                op=ADD,
            )
            nc.vector.reciprocal(rrecf[:], zred[:])
            nc.vector.tensor_copy(rrec4[0:16, :], rrecf[:])
            for tc4 in range(1, 4):
                eng = (nc.scalar, nc.sync, nc.scalar)[tc4 - 1]
                eng.dma_start(rrec4[32 * tc4 : 32 * tc4 + 16, :], rrec4[0:16, :])

            vt_cur = marshal_vt(0, load_wo=True)

            def emit_outproj(ctc, grp):
                for ob2 in range(4):
                    osb = opool.tile([128, 2 * G], F32, tag="osb")
                    for obh in range(2):
                        ob = 2 * ob2 + obh
                        pso = ps_o.tile([128, G], F32, tag="psO")
                        for m in range(8):
                            nc.tensor.matmul(
                                pso[:],
                                lhsT=wo_sb[
                                    :, m * C + ob * 128 : m * C + (ob + 1) * 128
                                ],
                                rhs=ctc[:, m * G : (m + 1) * G],
                                start=(m == 0),
                                stop=(m == 7),
                            )
                        nc.scalar.activation(
                            osb[:, obh * G : (obh + 1) * G],
                            pso[:],
                            Ident,
                            bias=boc_sb[:, ob : ob + 1],
                        )
                    nc.sync.dma_start(
                        out_d[
                            ob2 * 256 : (ob2 + 1) * 256, grp * G : (grp + 1) * G
                        ].rearrange("(obh p) t -> p obh t", obh=2),
                        osb[:].rearrange("p (obh t) -> p obh t", obh=2),
                    )

            def emit_at(grp):
                # attn = E * (1/Z)
                at4 = atpool.tile([128, 2048], BF16, tag="at4")
                nc.vector.tensor_tensor(
                    out=at4[:].rearrange("p (sl h) -> p sl h", h=H),
                    in0=se_res[:, grp * 2048 : (grp + 1) * 2048].rearrange(
                        "p (sl h) -> p sl h", h=H
                    ),
                    in1=rrec4[:].unsqueeze(1).broadcast_to([128, 128, H]),
                    op=MUL,
                )
                return at4

            at_cur = emit_at(0)
            pend = None
            for grp in range(NG):
                vt4 = vt_cur
                at4 = at_cur
                if grp + 1 < NG:
                    vt_cur = marshal_vt(grp + 1)
                ctc = ctcpool.tile([128, 8 * G], BF16, tag="ctc")
                vtv = vt4[:].rearrange("p (d sl) -> p sl d", d=DK)
                a4v = at4[:].rearrange("p (sl h) -> p sl h", h=H)
                for q in range(8):
                    pcs = ps_c.tile([128, 512], F32, tag="psC")
                    for i64 in range(64):
                        i = q * 64 + i64
                        tc4, sl = i >> 7, i & 127
                        lw = vtv[32 * tc4 : 32 * tc4 + 16, sl, :]
                        for hp in range(2):
                            nc.tensor.matmul(
                                pcs[64 * hp : 64 * hp + 64, i64 * 8 : i64 * 8 + 8],
                                lhsT=lw,
                                rhs=a4v[
                                    32 * tc4 : 32 * tc4 + 16, sl, 8 * hp : 8 * hp + 8
                                ],
                                start=True,
                                stop=True,
                                tile_position=(32 * tc4, 64 * hp),
                            )
                    dst = ctc[:].rearrange("p (m t) -> p m t", m=8)[
                        :, :, q * 64 : (q + 1) * 64
                    ]
                    src = pcs[:].rearrange("p (sc m) -> p m sc", m=8)
                    if q % 2 == 0:
                        nc.vector.tensor_copy(dst, src)
                    else:
                        nc.scalar.activation(dst, src, Copy)
                if grp + 1 < NG:
                    at_cur = emit_at(grp + 1)
                if pend is not None:
                    emit_outproj(*pend)
                pend = (ctc, grp)
            emit_outproj(*pend)

    _split_sync_waits(nc, limit=1)
    return nc


_NC_CACHE = {}


def _get_nc(T, _span=None):
    key = T
    if key not in _NC_CACHE:
        _NC_CACHE[key] = build_kernel(T)
    return _NC_CACHE[key]


def _prep_weights(w_qkv, b_qkv, w_out, b_out):
    bf = ml_dtypes.bfloat16
    w3 = w_qkv.reshape(H, 192, C).astype(np.float32)
    qw = (w3[:, :DK, :] / 8.0).reshape(H * DK, C)
    kw = w3[:, DK : 2 * DK, :].reshape(H * DK, C)
    vw = w3[:, 2 * DK :, :].reshape(H * DK, C)
    # mtile order: m 0..7 = Q head pairs, 8..15 = K head pairs, 16..23 = V.
    wqT = np.concatenate([qw, kw, vw], axis=0).T.copy().astype(bf)  # (C, 3072)
    b3 = b_qkv.reshape(H, 192).astype(np.float32)
    bq_all = np.concatenate(
        [
            (b3[:, :DK] / 8.0).reshape(-1),
            b3[:, DK : 2 * DK].reshape(-1),
            b3[:, 2 * DK :].reshape(-1),
        ]
    )
    bqc = bq_all.reshape(24, 128).T.copy().astype(np.float32)  # [128, 24]
    # wo2[hp*64+d, m*C+o] = w_out[o, (hp*8+m)*64+d]
    wom = w_out.astype(np.float32).reshape(C, H, DK)  # [o, h, d]
    w4 = wom.transpose(1, 2, 0).reshape(2, 8, DK, C)  # [hp, m, d, o]
    wo2 = np.ascontiguousarray(w4.transpose(0, 2, 1, 3)).reshape(128, 8 * C).astype(bf)
    boc = np.ascontiguousarray(b_out.astype(np.float32).reshape(8, 128).T)  # [128, 8]
    return wqT, bqc, wo2, boc


def kernel(x, w_qkv, b_qkv, w_out, b_out, _trace=False, _span=None):
    B, _, T = x.shape
    assert B == N_CORES
    nc = _get_nc(T)
    wqT, bqc, wo2, boc = _prep_weights(w_qkv, b_qkv, w_out, b_out)
    bf = ml_dtypes.bfloat16
    in_maps = []
    for b in range(B):
        in_maps.append(
            {
                "x": x[b].astype(bf),
                "wqT": wqT,
                "bqc": bqc,
                "wo2": wo2,
                "boc": boc,
            }
        )
    res = run_bass_kernel_spmd(nc, in_maps, list(range(N_CORES)), trace=_trace)
    out = np.stack([res.results[b]["out"] for b in range(B)], axis=0)
    if _trace:
        kernel.last_exec_time_ns = res.exec_time_ns
        kernel.last_results = res
    return out.astype(np.float32)


# revision 41
# speedup vs baseline: 1.0095x; 1.0081x over previous
"""Trainium2 Bass kernel for nn_MultiHeadAttention_53463752900838.

Math (per batch element b, one NeuronCore each — pure data parallel over B=8):
  qkv = w_qkv @ x + b_qkv                     (3072, T)
  q,k,v per head h: (64, T);  q scaled by 1/8 (folded into weights on host)
  scores[t,h,g] = sum_d q[h,d,t] k[g,d,t]     per-timestep 16x16 Gram matrix
  attn = softmax over t  (per (h,g) pair)
  context[h,d,t] = sum_g attn[t,h,g] v[g,d,t]
  out = w_out @ context + b_out               (1024, T)

Layout strategy (all bf16 matmuls, fp32 PSUM):
  Phase 1 (per 256-t span): QKV projection in (o, t) orientation, bias folded
    into the PSUM evacuation (per-partition bias add on DVE/Act, no bias
    matmuls). Q/K marshaled by strided DMA into a [64, (kind, h, t)] tile
    (both at partition base 0). V evacuated into a full-T SBUF-resident
    tile (no DRAM spill). Per-t 16x16 scores matmuls write a 4-bank psum
    tile packed over partitions by 128-t blocks: partition (32*tc+g),
    free (slot*16+h) with t = grp*512 + tc*128 + slot. One fused-exp evac
    per 512 timesteps into the SBUF-resident E tile; running Z on DVE.
  Phase 2 (per 512-t chunk): attn = E * (1/Z) on DVE (2x mode, partition-
    aligned). Context computed TRANSPOSED per t with matching partition
    bases: lhsT = v_t[16 g, 64 d] and rhs = attn_t[16 g, 8 h] both at base
    32*tc, out at psum partitions (64*hp + d) via tile_position; cheap evac
    to channel-major CTC; output projection accumulates over 8 head-pairs,
    bias via activation evac, written straight to (C, T) f32 output.
"""

import os
import sys
import contextlib

import numpy as np
import ml_dtypes

for p in ("/opt/trn_rl_repo",):
    if p not in sys.path and os.path.isdir(p):
        sys.path.insert(0, p)

import concourse.bass as bass
import concourse.tile as tile
from concourse import mybir
from concourse.bass_utils import run_bass_kernel_spmd

F32 = mybir.dt.float32
BF16 = mybir.dt.bfloat16

N_CORES = 8
C = 1024
H = 16
DK = 64
OC3 = 3072


def _split_sync_waits(nc, limit=1):
    """walrus codegen rejects too many semaphore waits per instruction (CTRL
    class takes 1); hoist overflow waits onto NoOps inserted before the
    offending instruction."""
    counter = [0]
    n_split = 0
    for fn in nc.m.functions:
        for bb in fn.blocks:
            out = []
            for ins in bb.instructions:
                si = getattr(ins, "sync_info", None)
                waits = list(si.on_wait) if (si is not None and si.on_wait) else []
                if len(waits) > limit:
                    n_split += 1
                    extra, keep = waits[:-limit], waits[-limit:]
                    for i in range(0, len(extra), limit):
                        counter[0] += 1
                        out.append(
                            mybir.InstNoOp(
                                name=f"I-wsplit-{counter[0]}",
                                opcode="NoOp",
                                engine=ins.engine,
                                ins=[],
                                outs=[],
                                sync_info=mybir.SyncInfo(
                                    on_wait=list(extra[i : i + limit]), on_update=[]
                                ),
                            )
                        )
                    si.on_wait = keep
                out.append(ins)
            bb.instructions[:] = out
    return n_split


def build_kernel(T=4096):
    S1 = 256              # phase-1 span (t per QKV round)
    NS1 = T // S1         # 16
    G = 512               # scores/phase-2 group size
    NG = T // G           # 8
    nc = bass.Bass("TRN2", target_bir_lowering=False, debug=False)

    x_in = nc.dram_tensor("x", [C, T], BF16, kind="ExternalInput").ap()
    wq_in = nc.dram_tensor("wqT", [C, OC3], BF16, kind="ExternalInput").ap()
    bqc_in = nc.dram_tensor("bqc", [128, 24], F32, kind="ExternalInput").ap()
    wo_in = nc.dram_tensor("wo2", [128, 8 * C], BF16, kind="ExternalInput").ap()
    boc_in = nc.dram_tensor("boc", [128, 8], F32, kind="ExternalInput").ap()
    out_d = nc.dram_tensor("out", [C, T], F32, kind="ExternalOutput").ap()

    Exp = mybir.ActivationFunctionType.Exp
    Ident = mybir.ActivationFunctionType.Identity
    Copy = mybir.ActivationFunctionType.Copy
    ADD = mybir.AluOpType.add
    MUL = mybir.AluOpType.mult
    AX = mybir.AxisListType.X

    with tile.TileContext(nc) as tc, contextlib.ExitStack() as octx:
        const = octx.enter_context(tc.tile_pool(name="const", bufs=1))
        bqc_sb = const.tile([128, 24], F32, tag="bqc")
        nc.sync.dma_start(bqc_sb[:], bqc_in)
        boc_sb = const.tile([128, 8], F32, tag="boc")
        nc.sync.dma_start(boc_sb[:], boc_in)
        # SBUF-resident V: [128=(gp*64+d), (j=8, T)] for g = 2j+gp
        v_res = const.tile([128, 8 * T], BF16, tag="vres")
        # SBUF-resident exp(scores): [128=(32*tc+g), (grp, slot, h)]
        # with t = grp*512 + tc*128 + slot
        se_res = const.tile([128, 4 * T], BF16, tag="seres")
        zfin = const.tile([16, 8 * 64], F32, tag="zfin")  # (grp, tc, h)
        zred = const.tile([16, 16], F32, tag="zred")
        rrecf = const.tile([16, 16], F32, tag="rrecf")
        rrec4 = const.tile([128, 16], BF16, tag="rrec4")
        nc.vector.memset(rrec4[:], 0.0)

        # ---------------- PHASE 1: QKV + scores + exp + Z ----------------
        with contextlib.ExitStack() as ctx:
            wpool = ctx.enter_context(tc.tile_pool(name="wq", bufs=1))
            xpool = ctx.enter_context(tc.tile_pool(name="xp", bufs=2))

            def load_x(s):
                # SWDGE path: never queued behind the wq loads on SP/HWDGE
                xa = xpool.tile([128, 8 * S1], BF16, tag="xa")
                nc.gpsimd.dma_start(
                    xa[:].rearrange("p (k t) -> p k t", k=8),
                    x_in[:, s * S1 : (s + 1) * S1].rearrange(
                        "(k p) t -> p k t", k=8
                    ),
                )
                return xa

            xa_next = load_x(0)
            # wq loaded in 8 column chunks of 3 mtiles each so the first
            # mtiles can start a few us in instead of waiting for all 6 MB;
            # the first chunk is split across SWDGE and HWDGE paths.
            CW = OC3 // 8  # 384
            wq_sb = []  # [chunk][k] -> [128, 384]
            for cch in range(8):
                row = []
                for k in range(8):
                    w = wpool.tile([128, CW], BF16, tag=f"wq{cch}_{k}")
                    eng = nc.gpsimd if (cch == 0 and k % 2 == 0) else nc.sync
                    eng.dma_start(
                        w[:], wq_in[k * 128 : (k + 1) * 128, cch * CW : (cch + 1) * CW]
                    )
                    row.append(w)
                wq_sb.append(row)

            stpool = ctx.enter_context(tc.tile_pool(name="st", bufs=2))
            qkpool = ctx.enter_context(tc.tile_pool(name="qkt", bufs=2))
            zpool = ctx.enter_context(tc.tile_pool(name="zp", bufs=3))
            ps_a = ctx.enter_context(tc.tile_pool(name="psA", bufs=4, space="PSUM"))
            ps_s = ctx.enter_context(tc.tile_pool(name="psS", bufs=1, space="PSUM"))

            # zero-fill the scores psum buffer once: partitions 32*tc+16..31
            # are never written by the 16-col matmuls but are read by the
            # wide evac.
            pstmp = ps_s.tile([128, 2048], F32, tag="psS")
            nc.vector.memset(pstmp[:], 0.0)

            def emit_scores(grp, qkts):
                # qkts: two qkt tiles covering spans (2*grp, 2*grp+1); each is
                # [64, (kind=2, h=16, t=S1)], kind 0 = Q, kind 1 = K.
                pss = ps_s.tile([128, 2048], F32, tag="psS")
                for tc4 in range(4):
                    qkt = qkts[tc4 >> 1]
                    qv = qkt[:].rearrange("d (kd h t) -> d kd h t", kd=2, h=H)
                    for sl in range(128):
                        t = (tc4 & 1) * 128 + sl
                        nc.tensor.matmul(
                            pss[32 * tc4 : 32 * tc4 + 16, sl * 16 : sl * 16 + 16],
                            lhsT=qv[:, 1, :, t],
                            rhs=qv[:, 0, :, t],
                            start=True,
                            stop=True,
                            tile_position=(0, 32 * tc4),
                        )
                # exp + Z-reduce in two halves so the tail-group critical
                # chain (exp -> reduce -> zfin -> ... -> first context matmul)
                # pipelines instead of serializing
                zth = [
                    zpool.tile([128, 16], F32, tag=f"zt{i}", name=f"zt{i}")
                    for i in range(2)
                ]
                for i in range(2):
                    seg = se_res[
                        :, grp * 2048 + i * 1024 : grp * 2048 + (i + 1) * 1024
                    ]
                    nc.scalar.activation(seg, pss[:, i * 1024 : (i + 1) * 1024], Exp)
                    nc.vector.tensor_reduce(
                        zth[i][:],
                        seg.rearrange("p (sl h) -> p h sl", h=H),
                        axis=AX,
                        op=ADD,
                    )
                zt = zpool.tile([128, 16], F32, tag="zts")
                nc.vector.tensor_tensor(
                    out=zt[:], in0=zth[0][:], in1=zth[1][:], op=ADD
                )
                last = grp == NG - 1
                for tc4 in range(4):
                    # Act (+SP for the last group): never head-of-line block
                    # the Pool marshal DMAs at the phase boundary
                    eng = (nc.scalar, nc.sync)[tc4 % 2] if last else nc.scalar
                    eng.dma_start(
                        zfin[
                            0:16, grp * 64 + tc4 * 16 : grp * 64 + (tc4 + 1) * 16
                        ],
                        zt[32 * tc4 : 32 * tc4 + 16, :],
                    )

            pending = []          # qkt tiles not yet consumed by emit_scores
            ngrp_done = 0
            for s in range(NS1):
                xa = xa_next
                st = stpool.tile([128, 16 * S1], BF16, tag="st")
                for m in range(24):
                    ps = ps_a.tile([128, S1], F32, tag="psA")
                    for k in range(8):
                        nc.tensor.matmul(
                            ps[:],
                            lhsT=wq_sb[m // 3][k][
                                :, (m % 3) * 128 : (m % 3 + 1) * 128
                            ],
                            rhs=xa[:, k * S1 : (k + 1) * S1],
                            start=(k == 0),
                            stop=(k == 7),
                        )
                    if m < 16:
                        dst = st[:, m * S1 : (m + 1) * S1]
                    else:
                        j = m - 16
                        dst = v_res[:, j * T + s * S1 : j * T + (s + 1) * S1]
                    if m % 2 == 0:
                        nc.vector.tensor_scalar(
                            dst, ps[:], bqc_sb[:, m : m + 1], None, ADD
                        )
                    else:
                        nc.scalar.activation(
                            dst, ps[:], Ident, bias=bqc_sb[:, m : m + 1]
                        )
                    # interleave scores of the previous group so the PE never
                    # waits on marshal DMAs
                    if m == 17 and len(pending) == 2 and s % 2 == 0:
                        emit_scores(ngrp_done, pending)
                        pending = []
                        ngrp_done += 1
                    if m == 11 and s + 1 < NS1:
                        xa_next = load_x(s + 1)
                    # marshal Q (m 0-7) as soon as its evacs are emitted, K
                    # (m 8-15) right after; qkt free = (kd*16+2*mm+hp)*S1+t =
                    # m*(2*S1) + hp*S1 + t with stage free (m, t).
                    if m == 7:
                        qkt = qkpool.tile([64, 2 * H * S1], BF16, tag="qkt")
                    if m in (7, 15):
                        mlo = 0 if m == 7 else 8
                        for hp in range(2):
                            nc.sync.dma_start(
                                qkt[:].rearrange(
                                    "d (m hp t) -> hp d m t", m=16, hp=2
                                )[hp, :, mlo : mlo + 8, :],
                                st[hp * 64 : (hp + 1) * 64, :].rearrange(
                                    "d (m t) -> d m t", m=16
                                )[:, mlo : mlo + 8, :],
                            )
                pending.append(qkt)
            while pending:
                emit_scores(ngrp_done, pending[:2])
                pending = pending[2:]
                ngrp_done += 1

        # ---------------- PHASE 2: attn * V + output projection ----------------
        with contextlib.ExitStack() as ctx:
            wopool = ctx.enter_context(tc.tile_pool(name="wo", bufs=1))
            wo_sb = wopool.tile([128, 8 * C], BF16, tag="wo")

            vtpool = ctx.enter_context(tc.tile_pool(name="vt", bufs=3))
            atpool = ctx.enter_context(tc.tile_pool(name="at", bufs=2))
            ctcpool = ctx.enter_context(tc.tile_pool(name="ctc", bufs=2))
            opool = ctx.enter_context(tc.tile_pool(name="osb", bufs=3))
            ps_c = ctx.enter_context(tc.tile_pool(name="psC", bufs=3, space="PSUM"))
            ps_o = ctx.enter_context(tc.tile_pool(name="psO", bufs=2, space="PSUM"))

            def marshal_vt(grp, load_wo=False):
                # vt4[32*tc + 2j+gp, (d, slot)] = v[g=2j+gp, d,
                # t = grp*512 + tc*128 + slot]
                vt4 = vtpool.tile([128, DK * 128], BF16, tag="vt4")
                for tc4 in range(4):
                    for j in range(8):
                        dst = vt4[
                            32 * tc4 + 2 * j : 32 * tc4 + 2 * j + 2, :
                        ].rearrange("p (d sl) -> p d sl", d=DK)
                        src = v_res[
                            :,
                            j * T
                            + grp * G
                            + tc4 * 128 : j * T
                            + grp * G
                            + (tc4 + 1) * 128,
                        ]
                        eng = (nc.gpsimd, nc.sync, nc.scalar, nc.gpsimd,
                               nc.sync, nc.scalar, nc.gpsimd, nc.sync)[j]
                        eng.dma_start(dst, src)
                    if load_wo and tc4 > 0:
                        # split the 2 MB wo load so it never monopolizes the
                        # DMA engines during the phase-boundary critical chain
                        nc.sync.dma_start(
                            wo_sb[:, (tc4 - 1) * 2048 : tc4 * 2048],
                            wo_in[:, (tc4 - 1) * 2048 : tc4 * 2048],
                        )
                if load_wo:
                    nc.sync.dma_start(wo_sb[:, 3 * 2048 :], wo_in[:, 3 * 2048 :])
                return vt4

            # ---- finalize Z -> rrec4 (emitted FIRST so its instructions sit
            # at the head of every queue at the phase boundary) ----
            nc.vector.tensor_reduce(
                zred[:],
                zfin[0:16, :].rearrange("g (gt h) -> g h gt", h=H),
                axis=AX,
                op=ADD,
            )
            nc.vector.reciprocal(rrecf[:], zred[:])
            nc.vector.tensor_copy(rrec4[0:16, :], rrecf[:])
            for tc4 in range(1, 4):
                eng = (nc.scalar, nc.sync, nc.scalar)[tc4 - 1]
                eng.dma_start(rrec4[32 * tc4 : 32 * tc4 + 16, :], rrec4[0:16, :])

            vt_cur = marshal_vt(0, load_wo=True)

            def emit_outproj(ctc, grp):
                for ob2 in range(4):
                    osb = opool.tile([128, 2 * G], F32, tag="osb")
                    for obh in range(2):
                        ob = 2 * ob2 + obh
                        pso = ps_o.tile([128, G], F32, tag="psO")
                        for m in range(8):
                            nc.tensor.matmul(
                                pso[:],
                                lhsT=wo_sb[
                                    :, m * C + ob * 128 : m * C + (ob + 1) * 128
                                ],
                                rhs=ctc[:, m * G : (m + 1) * G],
                                start=(m == 0),
                                stop=(m == 7),
                            )
                        nc.scalar.activation(
                            osb[:, obh * G : (obh + 1) * G],
                            pso[:],
                            Ident,
                            bias=boc_sb[:, ob : ob + 1],
                        )
                    nc.sync.dma_start(
                        out_d[
                            ob2 * 256 : (ob2 + 1) * 256, grp * G : (grp + 1) * G
                        ].rearrange("(obh p) t -> p obh t", obh=2),
                        osb[:].rearrange("p (obh t) -> p obh t", obh=2),
                    )

            def emit_at(grp):
                # attn = E * (1/Z)
                at4 = atpool.tile([128, 2048], BF16, tag="at4")
                nc.vector.tensor_tensor(
                    out=at4[:].rearrange("p (sl h) -> p sl h", h=H),
                    in0=se_res[:, grp * 2048 : (grp + 1) * 2048].rearrange(
                        "p (sl h) -> p sl h", h=H
                    ),
                    in1=rrec4[:].unsqueeze(1).broadcast_to([128, 128, H]),
                    op=MUL,
                )
                return at4

            at_cur = emit_at(0)
            pend = None
            for grp in range(NG):
                vt4 = vt_cur
                at4 = at_cur
                if grp + 1 < NG:
                    vt_cur = marshal_vt(grp + 1)
                ctc = ctcpool.tile([128, 8 * G], BF16, tag="ctc")
                vtv = vt4[:].rearrange("p (d sl) -> p sl d", d=DK)
                a4v = at4[:].rearrange("p (sl h) -> p sl h", h=H)
                for q in range(8):
                    pcs = ps_c.tile([128, 512], F32, tag="psC")
                    for i64 in range(64):
                        i = q * 64 + i64
                        tc4, sl = i >> 7, i & 127
                        lw = vtv[32 * tc4 : 32 * tc4 + 16, sl, :]
                        for hp in range(2):
                            nc.tensor.matmul(
                                pcs[64 * hp : 64 * hp + 64, i64 * 8 : i64 * 8 + 8],
                                lhsT=lw,
                                rhs=a4v[
                                    32 * tc4 : 32 * tc4 + 16, sl, 8 * hp : 8 * hp + 8
                                ],
                                start=True,
                                stop=True,
                                tile_position=(32 * tc4, 64 * hp),
                            )
                    dst = ctc[:].rearrange("p (m t) -> p m t", m=8)[
                        :, :, q * 64 : (q + 1) * 64
                    ]
                    src = pcs[:].rearrange("p (sc m) -> p m sc", m=8)
                    if q % 2 == 0:
                        nc.vector.tensor_copy(dst, src)
                    else:
                        nc.scalar.activation(dst, src, Copy)
                if grp + 1 < NG:
                    at_cur = emit_at(grp + 1)
                if pend is not None:
                    emit_outproj(*pend)
                pend = (ctc, grp)
            emit_outproj(*pend)

    _split_sync_waits(nc, limit=1)
    return nc


_NC_CACHE = {}


def _get_nc(T, _span=None):
    key = T
    if key not in _NC_CACHE:
        _NC_CACHE[key] = build_kernel(T)
    return _NC_CACHE[key]


def _prep_weights(w_qkv, b_qkv, w_out, b_out):
    bf = ml_dtypes.bfloat16
    w3 = w_qkv.reshape(H, 192, C).astype(np.float32)
    qw = (w3[:, :DK, :] / 8.0).reshape(H * DK, C)
    kw = w3[:, DK : 2 * DK, :].reshape(H * DK, C)
    vw = w3[:, 2 * DK :, :].reshape(H * DK, C)
    # mtile order: m 0..7 = Q head pairs, 8..15 = K head pairs, 16..23 = V.
    wqT = np.concatenate([qw, kw, vw], axis=0).T.copy().astype(bf)  # (C, 3072)
    b3 = b_qkv.reshape(H, 192).astype(np.float32)
    bq_all = np.concatenate(
        [
            (b3[:, :DK] / 8.0).reshape(-1),
            b3[:, DK : 2 * DK].reshape(-1),
            b3[:, 2 * DK :].reshape(-1),
        ]
    )
    bqc = bq_all.reshape(24, 128).T.copy().astype(np.float32)  # [128, 24]
    # wo2[hp*64+d, m*C+o] = w_out[o, (hp*8+m)*64+d]
    wom = w_out.astype(np.float32).reshape(C, H, DK)  # [o, h, d]
    w4 = wom.transpose(1, 2, 0).reshape(2, 8, DK, C)  # [hp, m, d, o]
    wo2 = np.ascontiguousarray(w4.transpose(0, 2, 1, 3)).reshape(128, 8 * C).astype(bf)
    boc = np.ascontiguousarray(b_out.astype(np.float32).reshape(8, 128).T)  # [128, 8]
    return wqT, bqc, wo2, boc


def kernel(x, w_qkv, b_qkv, w_out, b_out, _trace=False, _span=None):
    B, _, T = x.shape
    assert B == N_CORES
    nc = _get_nc(T)
    wqT, bqc, wo2, boc = _prep_weights(w_qkv, b_qkv, w_out, b_out)
    bf = ml_dtypes.bfloat16
    in_maps = []
    for b in range(B):
        in_maps.append(
            {
                "x": x[b].astype(bf),
                "wqT": wqT,
                "bqc": bqc,
                "wo2": wo2,
                "boc": boc,
            }
        )
    res = run_bass_kernel_spmd(nc, in_maps, list(range(N_CORES)), trace=_trace)
    out = np.stack([res.results[b]["out"] for b in range(B)], axis=0)
    if _trace:
        kernel.last_exec_time_ns = res.exec_time_ns
        kernel.last_results = res
    return out.astype(np.float32)


# revision 44
# speedup vs baseline: 1.0112x; 1.0016x over previous
"""Trainium2 Bass kernel for nn_MultiHeadAttention_53463752900838.

Math (per batch element b, one NeuronCore each — pure data parallel over B=8):
  qkv = w_qkv @ x + b_qkv                     (3072, T)
  q,k,v per head h: (64, T);  q scaled by 1/8 (folded into weights on host)
  scores[t,h,g] = sum_d q[h,d,t] k[g,d,t]     per-timestep 16x16 Gram matrix
  attn = softmax over t  (per (h,g) pair)
  context[h,d,t] = sum_g attn[t,h,g] v[g,d,t]
  out = w_out @ context + b_out               (1024, T)

Layout strategy (all bf16 matmuls, fp32 PSUM):
  Phase 1 (per 256-t span): QKV projection in (o, t) orientation, bias folded
    into the PSUM evacuation (per-partition bias add on DVE/Act, no bias
    matmuls). Q/K marshaled by strided DMA into a [64, (kind, h, t)] tile
    (both at partition base 0). V evacuated into a full-T SBUF-resident
    tile (no DRAM spill). Per-t 16x16 scores matmuls write a 4-bank psum
    tile packed over partitions by 128-t blocks: partition (32*tc+g),
    free (slot*16+h) with t = grp*512 + tc*128 + slot. One fused-exp evac
    per 512 timesteps into the SBUF-resident E tile; running Z on DVE.
  Phase 2 (per 512-t chunk): attn = E * (1/Z) on DVE (2x mode, partition-
    aligned). Context computed TRANSPOSED per t with matching partition
    bases: lhsT = v_t[16 g, 64 d] and rhs = attn_t[16 g, 8 h] both at base
    32*tc, out at psum partitions (64*hp + d) via tile_position; cheap evac
    to channel-major CTC; output projection accumulates over 8 head-pairs,
    bias via activation evac, written straight to (C, T) f32 output.
"""

import os
import sys
import contextlib

import numpy as np
import ml_dtypes

for p in ("/opt/trn_rl_repo",):
    if p not in sys.path and os.path.isdir(p):
        sys.path.insert(0, p)

import concourse.bass as bass
import concourse.tile as tile
from concourse import mybir
from concourse.bass_utils import run_bass_kernel_spmd

F32 = mybir.dt.float32
BF16 = mybir.dt.bfloat16

N_CORES = 8
C = 1024
H = 16
DK = 64
OC3 = 3072


def _split_sync_waits(nc, limit=1):
    """walrus codegen rejects too many semaphore waits per instruction (CTRL
    class takes 1); hoist overflow waits onto NoOps inserted before the
    offending instruction."""
    counter = [0]
    n_split = 0
    for fn in nc.m.functions:
        for bb in fn.blocks:
            out = []
            for ins in bb.instructions:
                si = getattr(ins, "sync_info", None)
                waits = list(si.on_wait) if (si is not None and si.on_wait) else []
                if len(waits) > limit:
                    n_split += 1
                    extra, keep = waits[:-limit], waits[-limit:]
                    for i in range(0, len(extra), limit):
                        counter[0] += 1
                        out.append(
                            mybir.InstNoOp(
                                name=f"I-wsplit-{counter[0]}",
                                opcode="NoOp",
                                engine=ins.engine,
                                ins=[],
                                outs=[],
                                sync_info=mybir.SyncInfo(
                                    on_wait=list(extra[i : i + limit]), on_update=[]
                                ),
                            )
                        )
                    si.on_wait = keep
                out.append(ins)
            bb.instructions[:] = out
    return n_split


def build_kernel(T=4096):
    S1 = 256              # phase-1 span (t per QKV round)
    NS1 = T // S1         # 16
    G = 512               # scores/phase-2 group size
    NG = T // G           # 8
    nc = bass.Bass("TRN2", target_bir_lowering=False, debug=False)

    x_in = nc.dram_tensor("x", [C, T], BF16, kind="ExternalInput").ap()
    wq_in = nc.dram_tensor("wqT", [C, OC3], BF16, kind="ExternalInput").ap()
    bqc_in = nc.dram_tensor("bqc", [128, 24], F32, kind="ExternalInput").ap()
    wo_in = nc.dram_tensor("wo2", [128, 8 * C], BF16, kind="ExternalInput").ap()
    boc_in = nc.dram_tensor("boc", [128, 8], F32, kind="ExternalInput").ap()
    out_d = nc.dram_tensor("out", [C, T], F32, kind="ExternalOutput").ap()

    Exp = mybir.ActivationFunctionType.Exp
    Ident = mybir.ActivationFunctionType.Identity
    Copy = mybir.ActivationFunctionType.Copy
    ADD = mybir.AluOpType.add
    MUL = mybir.AluOpType.mult
    AX = mybir.AxisListType.X

    with tile.TileContext(nc) as tc, contextlib.ExitStack() as octx:
        const = octx.enter_context(tc.tile_pool(name="const", bufs=1))
        bqc_sb = const.tile([128, 24], F32, tag="bqc")
        nc.sync.dma_start(bqc_sb[:], bqc_in)
        boc_sb = const.tile([128, 8], F32, tag="boc")
        nc.sync.dma_start(boc_sb[:], boc_in)
        # SBUF-resident V: [128=(gp*64+d), (j=8, T)] for g = 2j+gp
        v_res = const.tile([128, 8 * T], BF16, tag="vres")
        # SBUF-resident exp(scores): [128=(32*tc+g), (grp, slot, h)]
        # with t = grp*512 + tc*128 + slot
        se_res = const.tile([128, 4 * T], BF16, tag="seres")
        zfin = const.tile([16, 8 * 64], F32, tag="zfin")  # (grp, tc, h)
        zred = const.tile([16, 16], F32, tag="zred")
        rrecf = const.tile([16, 16], F32, tag="rrecf")
        rrec4 = const.tile([128, 16], BF16, tag="rrec4")
        nc.vector.memset(rrec4[:], 0.0)

        # ---------------- PHASE 1: QKV + scores + exp + Z ----------------
        with contextlib.ExitStack() as ctx:
            wpool = ctx.enter_context(tc.tile_pool(name="wq", bufs=1))
            xpool = ctx.enter_context(tc.tile_pool(name="xp", bufs=2))

            def load_x(s):
                # SWDGE path: never queued behind the wq loads on SP/HWDGE
                xa = xpool.tile([128, 8 * S1], BF16, tag="xa")
                nc.gpsimd.dma_start(
                    xa[:].rearrange("p (k t) -> p k t", k=8),
                    x_in[:, s * S1 : (s + 1) * S1].rearrange(
                        "(k p) t -> p k t", k=8
                    ),
                )
                return xa

            xa_next = load_x(0)
            # wq loaded in 8 column chunks of 3 mtiles each so the first
            # mtiles can start a few us in instead of waiting for all 6 MB;
            # the first chunk is split across SWDGE and HWDGE paths.
            CW = OC3 // 8  # 384
            wq_sb = []  # [chunk][k] -> [128, 384]
            for cch in range(8):
                row = []
                for k in range(8):
                    w = wpool.tile([128, CW], BF16, tag=f"wq{cch}_{k}")
                    eng = nc.gpsimd if (cch == 0 and k % 2 == 0) else nc.sync
                    eng.dma_start(
                        w[:], wq_in[k * 128 : (k + 1) * 128, cch * CW : (cch + 1) * CW]
                    )
                    row.append(w)
                wq_sb.append(row)

            stpool = ctx.enter_context(tc.tile_pool(name="st", bufs=2))
            qkpool = ctx.enter_context(tc.tile_pool(name="qkt", bufs=2))
            zpool = ctx.enter_context(tc.tile_pool(name="zp", bufs=3))
            ps_a = ctx.enter_context(tc.tile_pool(name="psA", bufs=4, space="PSUM"))
            ps_s = ctx.enter_context(tc.tile_pool(name="psS", bufs=1, space="PSUM"))

            # zero-fill the scores psum buffer once: partitions 32*tc+16..31
            # are never written by the 16-col matmuls but are read by the
            # wide evac.
            pstmp = ps_s.tile([128, 2048], F32, tag="psS")
            nc.vector.memset(pstmp[:], 0.0)

            def emit_scores(grp, qkts, half=None, pss=None):
                # qkts: qkt tiles [64, (kind=2, h=16, t=S1)], kind 0=Q, 1=K.
                # half=None: both spans (2*grp, 2*grp+1); half=0/1: only the
                # tc-pair of one span (partition-split tail group).
                if pss is None:
                    pss = ps_s.tile([128, 2048], F32, tag="psS")
                tcs = (0, 1, 2, 3) if half is None else ((0, 1), (2, 3))[half]
                for tc4 in tcs:
                    qkt = qkts[tc4 >> 1] if half is None else qkts[0]
                    qv = qkt[:].rearrange("d (kd h t) -> d kd h t", kd=2, h=H)
                    for sl in range(128):
                        t = (tc4 & 1) * 128 + sl
                        nc.tensor.matmul(
                            pss[32 * tc4 : 32 * tc4 + 16, sl * 16 : sl * 16 + 16],
                            lhsT=qv[:, 1, :, t],
                            rhs=qv[:, 0, :, t],
                            start=True,
                            stop=True,
                            tile_position=(0, 32 * tc4),
                        )
                # exp + Z-reduce in two sl-halves so the tail-group critical
                # chain (exp -> reduce -> zfin -> ... -> first context matmul)
                # pipelines instead of serializing
                p0 = 0 if half is None else 64 * half
                np_ = 128 if half is None else 64
                zth = [
                    zpool.tile([128, 16], F32, tag=f"zt{i}", name=f"zt{i}")
                    for i in range(2)
                ]
                for i in range(2):
                    seg = se_res[
                        p0 : p0 + np_,
                        grp * 2048 + i * 1024 : grp * 2048 + (i + 1) * 1024,
                    ]
                    nc.scalar.activation(
                        seg, pss[p0 : p0 + np_, i * 1024 : (i + 1) * 1024], Exp
                    )
                    nc.vector.tensor_reduce(
                        zth[i][p0 : p0 + np_, :],
                        seg.rearrange("p (sl h) -> p h sl", h=H),
                        axis=AX,
                        op=ADD,
                    )
                zt = zpool.tile([128, 16], F32, tag="zts")
                nc.vector.tensor_tensor(
                    out=zt[p0 : p0 + np_, :],
                    in0=zth[0][p0 : p0 + np_, :],
                    in1=zth[1][p0 : p0 + np_, :],
                    op=ADD,
                )
                last = grp == NG - 1
                for tc4 in tcs:
                    # Act (+SP for the last group): never head-of-line block
                    # the Pool marshal DMAs at the phase boundary
                    eng = (nc.scalar, nc.sync)[tc4 % 2] if last else nc.scalar
                    eng.dma_start(
                        zfin[
                            0:16, grp * 64 + tc4 * 16 : grp * 64 + (tc4 + 1) * 16
                        ],
                        zt[32 * tc4 : 32 * tc4 + 16, :],
                    )
                return pss

            pending = []          # qkt tiles not yet consumed by emit_scores
            ngrp_done = 0
            for s in range(NS1):
                xa = xa_next
                st = stpool.tile([128, 16 * S1], BF16, tag="st")
                for m in range(24):
                    ps = ps_a.tile([128, S1], F32, tag="psA")
                    for k in range(8):
                        nc.tensor.matmul(
                            ps[:],
                            lhsT=wq_sb[m // 3][k][
                                :, (m % 3) * 128 : (m % 3 + 1) * 128
                            ],
                            rhs=xa[:, k * S1 : (k + 1) * S1],
                            start=(k == 0),
                            stop=(k == 7),
                        )
                    if m < 16:
                        dst = st[:, m * S1 : (m + 1) * S1]
                    else:
                        j = m - 16
                        dst = v_res[:, j * T + s * S1 : j * T + (s + 1) * S1]
                    if m % 2 == 0:
                        nc.vector.tensor_scalar(
                            dst, ps[:], bqc_sb[:, m : m + 1], None, ADD
                        )
                    else:
                        nc.scalar.activation(
                            dst, ps[:], Ident, bias=bqc_sb[:, m : m + 1]
                        )
                    # interleave scores of the previous group so the PE never
                    # waits on marshal DMAs
                    if m == 17 and len(pending) == 2 and s % 2 == 0:
                        emit_scores(ngrp_done, pending)
                        pending = []
                        ngrp_done += 1
                    if m == 17 and s == NS1 - 1:
                        # tail group: emit the span-14 half now so only half
                        # of its chain remains after the last span
                        pss_tail = emit_scores(ngrp_done, pending, half=0)
                    if m == 11 and s + 1 < NS1:
                        xa_next = load_x(s + 1)
                    # marshal Q (m 0-7) as soon as its evacs are emitted, K
                    # (m 8-15) right after; qkt free = (kd*16+2*mm+hp)*S1+t =
                    # m*(2*S1) + hp*S1 + t with stage free (m, t).
                    if m == 7:
                        qkt = qkpool.tile([64, 2 * H * S1], BF16, tag="qkt")
                    if m in (7, 15):
                        mlo = 0 if m == 7 else 8
                        for hp in range(2):
                            nc.sync.dma_start(
                                qkt[:].rearrange(
                                    "d (m hp t) -> hp d m t", m=16, hp=2
                                )[hp, :, mlo : mlo + 8, :],
                                st[hp * 64 : (hp + 1) * 64, :].rearrange(
                                    "d (m t) -> d m t", m=16
                                )[:, mlo : mlo + 8, :],
                            )
                pending.append(qkt)
            emit_scores(ngrp_done, [pending[1]], half=1, pss=pss_tail)

        # ---------------- PHASE 2: attn * V + output projection ----------------
        with contextlib.ExitStack() as ctx:
            wopool = ctx.enter_context(tc.tile_pool(name="wo", bufs=1))
            wo_sb = wopool.tile([128, 8 * C], BF16, tag="wo")

            vtpool = ctx.enter_context(tc.tile_pool(name="vt", bufs=3))
            atpool = ctx.enter_context(tc.tile_pool(name="at", bufs=2))
            ctcpool = ctx.enter_context(tc.tile_pool(name="ctc", bufs=2))
            opool = ctx.enter_context(tc.tile_pool(name="osb", bufs=3))
            ps_c = ctx.enter_context(tc.tile_pool(name="psC", bufs=3, space="PSUM"))
            ps_o = ctx.enter_context(tc.tile_pool(name="psO", bufs=2, space="PSUM"))

            def marshal_vt(grp, load_wo=False):
                # vt4[32*tc + 2j+gp, (d, slot)] = v[g=2j+gp, d,
                # t = grp*512 + tc*128 + slot]
                vt4 = vtpool.tile([128, DK * 128], BF16, tag="vt4")
                for tc4 in range(4):
                    for j in range(8):
                        dst = vt4[
                            32 * tc4 + 2 * j : 32 * tc4 + 2 * j + 2, :
                        ].rearrange("p (d sl) -> p d sl", d=DK)
                        src = v_res[
                            :,
                            j * T
                            + grp * G
                            + tc4 * 128 : j * T
                            + grp * G
                            + (tc4 + 1) * 128,
                        ]
                        eng = (nc.gpsimd, nc.sync, nc.scalar, nc.gpsimd,
                               nc.sync, nc.scalar, nc.gpsimd, nc.sync)[j]
                        eng.dma_start(dst, src)
                    if load_wo and tc4 > 0:
                        # split the 2 MB wo load so it never monopolizes the
                        # DMA engines during the phase-boundary critical chain
                        nc.sync.dma_start(
                            wo_sb[:, (tc4 - 1) * 2048 : tc4 * 2048],
                            wo_in[:, (tc4 - 1) * 2048 : tc4 * 2048],
                        )
                if load_wo:
                    nc.sync.dma_start(wo_sb[:, 3 * 2048 :], wo_in[:, 3 * 2048 :])
                return vt4

            # ---- finalize Z -> rrec4 (emitted FIRST so its instructions sit
            # at the head of every queue at the phase boundary) ----
            nc.vector.tensor_reduce(
                zred[:],
                zfin[0:16, :].rearrange("g (gt h) -> g h gt", h=H),
                axis=AX,
                op=ADD,
            )
            nc.vector.reciprocal(rrecf[:], zred[:])
            nc.vector.tensor_copy(rrec4[0:16, :], rrecf[:])
            for tc4 in range(1, 4):
                eng = (nc.scalar, nc.sync, nc.scalar)[tc4 - 1]
                eng.dma_start(rrec4[32 * tc4 : 32 * tc4 + 16, :], rrec4[0:16, :])

            vt_cur = marshal_vt(0, load_wo=True)

            def emit_outproj(ctc, grp):
                for ob2 in range(4):
                    osb = opool.tile([128, 2 * G], F32, tag="osb")
                    for obh in range(2):
                        ob = 2 * ob2 + obh
                        pso = ps_o.tile([128, G], F32, tag="psO")
                        for m in range(8):
                            nc.tensor.matmul(
                                pso[:],
                                lhsT=wo_sb[
                                    :, m * C + ob * 128 : m * C + (ob + 1) * 128
                                ],
                                rhs=ctc[:, m * G : (m + 1) * G],
                                start=(m == 0),
                                stop=(m == 7),
                            )
                        nc.scalar.activation(
                            osb[:, obh * G : (obh + 1) * G],
                            pso[:],
                            Ident,
                            bias=boc_sb[:, ob : ob + 1],
                        )
                    nc.sync.dma_start(
                        out_d[
                            ob2 * 256 : (ob2 + 1) * 256, grp * G : (grp + 1) * G
                        ].rearrange("(obh p) t -> p obh t", obh=2),
                        osb[:].rearrange("p (obh t) -> p obh t", obh=2),
                    )

            def emit_at(grp):
                # attn = E * (1/Z)
                at4 = atpool.tile([128, 2048], BF16, tag="at4")
                nc.vector.tensor_tensor(
                    out=at4[:].rearrange("p (sl h) -> p sl h", h=H),
                    in0=se_res[:, grp * 2048 : (grp + 1) * 2048].rearrange(
                        "p (sl h) -> p sl h", h=H
                    ),
                    in1=rrec4[:].unsqueeze(1).broadcast_to([128, 128, H]),
                    op=MUL,
                )
                return at4

            at_cur = emit_at(0)
            pend = None
            for grp in range(NG):
                vt4 = vt_cur
                at4 = at_cur
                if grp + 1 < NG:
                    vt_cur = marshal_vt(grp + 1)
                ctc = ctcpool.tile([128, 8 * G], BF16, tag="ctc")
                vtv = vt4[:].rearrange("p (d sl) -> p sl d", d=DK)
                a4v = at4[:].rearrange("p (sl h) -> p sl h", h=H)
                for q in range(8):
                    pcs = ps_c.tile([128, 512], F32, tag="psC")
                    for i64 in range(64):
                        i = q * 64 + i64
                        tc4, sl = i >> 7, i & 127
                        lw = vtv[32 * tc4 : 32 * tc4 + 16, sl, :]
                        for hp in range(2):
                            nc.tensor.matmul(
                                pcs[64 * hp : 64 * hp + 64, i64 * 8 : i64 * 8 + 8],
                                lhsT=lw,
                                rhs=a4v[
                                    32 * tc4 : 32 * tc4 + 16, sl, 8 * hp : 8 * hp + 8
                                ],
                                start=True,
                                stop=True,
                                tile_position=(32 * tc4, 64 * hp),
                            )
                    dst = ctc[:].rearrange("p (m t) -> p m t", m=8)[
                        :, :, q * 64 : (q + 1) * 64
                    ]
                    src = pcs[:].rearrange("p (sc m) -> p m sc", m=8)
                    if q % 2 == 0:
                        nc.vector.tensor_copy(dst, src)
                    else:
                        nc.scalar.activation(dst, src, Copy)
                if grp + 1 < NG:
                    at_cur = emit_at(grp + 1)
                if pend is not None:
                    emit_outproj(*pend)
                pend = (ctc, grp)
            emit_outproj(*pend)

    _split_sync_waits(nc, limit=1)
    return nc


_NC_CACHE = {}


def _get_nc(T, _span=None):
    key = T
    if key not in _NC_CACHE:
        _NC_CACHE[key] = build_kernel(T)
    return _NC_CACHE[key]


def _prep_weights(w_qkv, b_qkv, w_out, b_out):
    bf = ml_dtypes.bfloat16
    w3 = w_qkv.reshape(H, 192, C).astype(np.float32)
    qw = (w3[:, :DK, :] / 8.0).reshape(H * DK, C)
    kw = w3[:, DK : 2 * DK, :].reshape(H * DK, C)
    vw = w3[:, 2 * DK :, :].reshape(H * DK, C)
    # mtile order: m 0..7 = Q head pairs, 8..15 = K head pairs, 16..23 = V.
    wqT = np.concatenate([qw, kw, vw], axis=0).T.copy().astype(bf)  # (C, 3072)
    b3 = b_qkv.reshape(H, 192).astype(np.float32)
    bq_all = np.concatenate(
        [
            (b3[:, :DK] / 8.0).reshape(-1),
            b3[:, DK : 2 * DK].reshape(-1),
            b3[:, 2 * DK :].reshape(-1),
        ]
    )
    bqc = bq_all.reshape(24, 128).T.copy().astype(np.float32)  # [128, 24]
    # wo2[hp*64+d, m*C+o] = w_out[o, (hp*8+m)*64+d]
    wom = w_out.astype(np.float32).reshape(C, H, DK)  # [o, h, d]
    w4 = wom.transpose(1, 2, 0).reshape(2, 8, DK, C)  # [hp, m, d, o]
    wo2 = np.ascontiguousarray(w4.transpose(0, 2, 1, 3)).reshape(128, 8 * C).astype(bf)
    boc = np.ascontiguousarray(b_out.astype(np.float32).reshape(8, 128).T)  # [128, 8]
    return wqT, bqc, wo2, boc


def kernel(x, w_qkv, b_qkv, w_out, b_out, _trace=False, _span=None):
    B, _, T = x.shape
    assert B == N_CORES
    nc = _get_nc(T)
    wqT, bqc, wo2, boc = _prep_weights(w_qkv, b_qkv, w_out, b_out)
    bf = ml_dtypes.bfloat16
    in_maps = []
    for b in range(B):
        in_maps.append(
            {
                "x": x[b].astype(bf),
                "wqT": wqT,
                "bqc": bqc,
                "wo2": wo2,
                "boc": boc,
            }
        )
    res = run_bass_kernel_spmd(nc, in_maps, list(range(N_CORES)), trace=_trace)
    out = np.stack([res.results[b]["out"] for b in range(B)], axis=0)
    if _trace:
        kernel.last_exec_time_ns = res.exec_time_ns
        kernel.last_results = res
    return out.astype(np.float32)
